# revision 5
# baseline (speedup 1.0000x reference)
"""Trainium2 Bass kernel for nn_ComposedFeatureTransformer (NNUE-style sparse
feature transformer / embedding lookup).

Computation (per feature set s in {0,1}):
    out_s[b] = bias + sum_k val_s[b,k] * W[idx_s[b,k]]      b in [0,8192), k in [0,32)
with W [45056, 2056] f32 (~370 MB), bias = concat(bias_ft[2048], bias_psqt[8]).

Strategy: data-parallel over the batch across 8 NeuronCores; the weight table is
replicated. Each core handles 1024 samples x 2 feature sets = 2048 rows, in 16
blocks of 128 samples. Per block:
  - rows W[idx[b,k]] are fetched with indirect (gathering) DMA, one row per
    SBUF partition, one k per DMA op ([128, 2056] f32, ~1 MB per op);
  - the weighted sum over k runs on the TensorEngine: stationary diag(val[:,k])
    (built by one DVE tensor_scalar_mul on a cached identity), moving operand =
    gathered rows, accumulated over all 32 k into PSUM [128, 2056] in
    one-PSUM-bank chunks of 512 columns;
  - bias is pre-seeded into PSUM via a K=1 matmul (ones[1,128].T @ bias[1,:]);
  - PSUM is copied to SBUF on the VectorEngine and DMA'd out.
"""

import os
import sys

import numpy as np

for _p in (
    "/root/.axon_site",
    "/root/.axon_site/_ro/trn_rl_repo",
    "/root/.axon_site/_ro/pypackages",
    "/opt/trn_rl_repo",
):
    if os.path.isdir(_p) and _p not in sys.path:
        sys.path.append(_p)

from contextlib import ExitStack

import concourse.bacc as bacc
import concourse.bass as bass
import concourse.tile as tile
from concourse import mybir
from concourse._compat import with_exitstack
from concourse.bass_utils import run_bass_kernel_spmd
from concourse.masks import make_identity

N_CORES = 8
NUM_INPUTS = 45056
L1 = 2048
NUM_PSQT = 8
D = L1 + NUM_PSQT            # 2056
BATCH = 8192
K = 32
BPC = BATCH // N_CORES       # 1024 samples per core per feature set
ROWS = 2 * BPC               # 2048 (set0 rows then set1 rows)
P = 128
NBLK = ROWS // P             # 16
CHUNK = 512                  # psum chunk = one PSUM bank of f32
# Blocks per For_i iteration. The loop back-edge resets semaphores; without it
# the ~16-incs-per-row indirect-gather completion semaphore overflows the
# 16-bit wait field (needs <= ~500 gathers per reset; 8 blocks = 256).
G = 8

# module-level knobs/results for the local test harness (harmless when unused)
TRACE = False
LAST_RESULTS = None

_cache: dict = {}


@with_exitstack
def _kernel_body(ctx: ExitStack, tc: tile.TileContext, idx_ap, val_ap, w_ap,
                 b_ap, out_ap):
    nc = tc.nc
    const = ctx.enter_context(tc.tile_pool(name="const", bufs=1))
    iv = ctx.enter_context(tc.tile_pool(name="iv", bufs=2))
    rows = ctx.enter_context(tc.tile_pool(name="rows", bufs=6))
    dpool = ctx.enter_context(tc.tile_pool(name="dpool", bufs=4))
    opool = ctx.enter_context(tc.tile_pool(name="opool", bufs=2))
    psum = ctx.enter_context(tc.tile_pool(name="psum", bufs=1, space="PSUM"))

    ident = const.tile([P, P], mybir.dt.float32)
    make_identity(nc, ident[:])
    ones = const.tile([1, P], mybir.dt.float32)
    nc.vector.memset(ones[:], 1.0)
    bias_sb = const.tile([1, D], mybir.dt.float32)
    nc.sync.dma_start(out=bias_sb[:1, :], in_=b_ap[None, :])

    nchunks = (D + CHUNK - 1) // CHUNK
    with tc.For_i(0, ROWS, G * P) as row0:
        for blk in range(G):
            bs = bass.ds(row0 + blk * P, P)
            idxb = iv.tile([P, K], mybir.dt.int32)
            nc.sync.dma_start(out=idxb[:], in_=idx_ap[bs, :])
            valb = iv.tile([P, K], mybir.dt.float32)
            nc.sync.dma_start(out=valb[:], in_=val_ap[bs, :])

            ps = psum.tile([P, D], mybir.dt.float32)
            for c in range(nchunks):
                cs = slice(c * CHUNK, min((c + 1) * CHUNK, D))
                nc.tensor.matmul(ps[:, cs], lhsT=ones[:, :], rhs=bias_sb[:1, cs],
                                 start=True, stop=False)

            for k in range(K):
                r = rows.tile([P, D], mybir.dt.float32)
                nc.gpsimd.indirect_dma_start(
                    out=r[:],
                    out_offset=None,
                    in_=w_ap[:],
                    in_offset=bass.IndirectOffsetOnAxis(ap=idxb[:, k:k + 1],
                                                        axis=0),
                )
                d = dpool.tile([P, P], mybir.dt.float32)
                nc.vector.tensor_scalar_mul(d[:], ident[:], valb[:, k:k + 1])
                for c in range(nchunks):
                    cs = slice(c * CHUNK, min((c + 1) * CHUNK, D))
                    nc.tensor.matmul(ps[:, cs], lhsT=d[:], rhs=r[:, cs],
                                     start=False, stop=(k == K - 1))

            outb = opool.tile([P, D], mybir.dt.float32)
            nc.vector.tensor_copy(outb[:], ps[:])
            nc.sync.dma_start(out=out_ap[bs, :], in_=outb[:])


def _build():
    nc = bacc.Bacc("TRN2", target_bir_lowering=False, debug=False)
    idx_t = nc.dram_tensor("idx", [ROWS, K], mybir.dt.int32,
                           kind="ExternalInput").ap()
    val_t = nc.dram_tensor("val", [ROWS, K], mybir.dt.float32,
                           kind="ExternalInput").ap()
    w_t = nc.dram_tensor("w", [NUM_INPUTS, D], mybir.dt.float32,
                         kind="ExternalInput").ap()
    b_t = nc.dram_tensor("bias", [D], mybir.dt.float32,
                         kind="ExternalInput").ap()
    out_t = nc.dram_tensor("out", [ROWS, D], mybir.dt.float32,
                           kind="ExternalOutput").ap()
    with tile.TileContext(nc) as tc:
        _kernel_body(tc, idx_t, val_t, w_t, b_t, out_t)
    nc.compile()
    return nc


def prepare(feature_indices_0, feature_values_0, feature_indices_1,
            feature_values_1, weight, bias_ft, bias_psqt):
    """Build (cached) program + per-core input maps."""
    idx0 = np.ascontiguousarray(np.asarray(feature_indices_0, dtype=np.int32))
    val0 = np.ascontiguousarray(np.asarray(feature_values_0, dtype=np.float32))
    idx1 = np.ascontiguousarray(np.asarray(feature_indices_1, dtype=np.int32))
    val1 = np.ascontiguousarray(np.asarray(feature_values_1, dtype=np.float32))
    w = np.ascontiguousarray(np.asarray(weight, dtype=np.float32))
    bias = np.concatenate([
        np.asarray(bias_ft, dtype=np.float32).ravel(),
        np.asarray(bias_psqt, dtype=np.float32).ravel(),
    ])

    if "nc" not in _cache:
        _cache["nc"] = _build()
    nc = _cache["nc"]

    in_maps = []
    for c in range(N_CORES):
        sl = slice(c * BPC, (c + 1) * BPC)
        in_maps.append({
            "idx": np.concatenate([idx0[sl], idx1[sl]], axis=0),
            "val": np.concatenate([val0[sl], val1[sl]], axis=0),
            "w": w,
            "bias": bias,
        })
    return nc, in_maps


def kernel(feature_indices_0, feature_values_0, feature_indices_1,
           feature_values_1, weight, bias_ft, bias_psqt):
    global LAST_RESULTS
    nc, in_maps = prepare(feature_indices_0, feature_values_0,
                          feature_indices_1, feature_values_1,
                          weight, bias_ft, bias_psqt)
    res = run_bass_kernel_spmd(nc, in_maps, core_ids=list(range(N_CORES)))
    LAST_RESULTS = res
    outs = [r["out"] for r in res.results]
    out0 = np.concatenate([o[:BPC] for o in outs], axis=0)
    out1 = np.concatenate([o[BPC:] for o in outs], axis=0)
    return out0, out1


# revision 9
# speedup vs baseline: 3.5693x; 3.5693x over previous
"""Trainium2 Bass kernel for nn_ComposedFeatureTransformer (NNUE-style sparse
feature transformer / embedding lookup).

Computation (per feature set s in {0,1}):
    out_s[b] = bias + sum_k val_s[b,k] * W[idx_s[b,k]]      b in [0,8192), k in [0,32)
with W [45056, 2056] f32 (~370 MB), bias = concat(bias_ft[2048], bias_psqt[8]).

Strategy: data-parallel over the batch across 8 NeuronCores; the weight table is
replicated. Each core handles 1024 samples x 2 feature sets = 2048 rows, in 16
blocks of 128 samples. Per block:
  - rows W[idx[b,k]] are fetched with indirect (gathering) DMA, one row per
    SBUF partition, one k per DMA op ([128, 2056] f32, ~1 MB per op);
  - the weighted sum over k runs on the Scalar (ACT) + Vector (DVE) engines:
    ACT computes tmp = r * val[:,k] (per-partition scale), DVE accumulates
    acc += tmp into an SBUF accumulator pre-initialized with the bias
    (broadcast across partitions once at startup via a K=1 PE matmul);
  - the accumulator is DMA'd out per block.
The batch loop is a For_i dynamic loop (8 blocks per iteration) so the loop
back-edge resets semaphores -- statically unrolling all 512 gathers overflows
the 16-bit semaphore wait field of the gather-completion semaphore.
"""

import os
import sys

import numpy as np

for _p in (
    "/root/.axon_site",
    "/root/.axon_site/_ro/trn_rl_repo",
    "/root/.axon_site/_ro/pypackages",
    "/opt/trn_rl_repo",
):
    if os.path.isdir(_p) and _p not in sys.path:
        sys.path.append(_p)

from contextlib import ExitStack

import concourse.bacc as bacc
import concourse.bass as bass
import concourse.tile as tile
from concourse import mybir
from concourse._compat import with_exitstack
from concourse.bass_utils import run_bass_kernel_spmd

N_CORES = 8
NUM_INPUTS = 45056
L1 = 2048
NUM_PSQT = 8
D = L1 + NUM_PSQT            # 2056
BATCH = 8192
K = 32
BPC = BATCH // N_CORES       # 1024 samples per core per feature set
ROWS = 2 * BPC               # 2048 (set0 rows then set1 rows)
P = 128
NBLK = ROWS // P             # 16
CHUNK = 512                  # psum chunk = one PSUM bank of f32
# Blocks per For_i iteration. The loop back-edge resets semaphores; without it
# the ~16-incs-per-row indirect-gather completion semaphore overflows the
# 16-bit wait field (needs <= ~500 gathers per reset; 8 blocks = 256).
G = 8

# module-level knobs/results for the local test harness (harmless when unused)
TRACE = False
LAST_RESULTS = None

_cache: dict = {}


@with_exitstack
def _kernel_body(ctx: ExitStack, tc: tile.TileContext, idx_ap, val_ap, w_ap,
                 b_ap, out_ap, rep=1):
    nc = tc.nc
    const = ctx.enter_context(tc.tile_pool(name="const", bufs=1))
    iv = ctx.enter_context(tc.tile_pool(name="iv", bufs=2))
    rows = ctx.enter_context(tc.tile_pool(name="rows", bufs=10))
    tpool = ctx.enter_context(tc.tile_pool(name="tpool", bufs=6))
    opool = ctx.enter_context(tc.tile_pool(name="opool", bufs=2))
    psum = ctx.enter_context(tc.tile_pool(name="psum", bufs=1, space="PSUM"))

    ones = const.tile([1, P], mybir.dt.float32)
    nc.vector.memset(ones[:], 1.0)
    bias_sb = const.tile([1, D], mybir.dt.float32)
    nc.sync.dma_start(out=bias_sb[:1, :], in_=b_ap[None, :])

    nchunks = (D + CHUNK - 1) // CHUNK
    # broadcast bias across partitions once: psum = ones.T @ bias, copy to SBUF
    bias_bcast = const.tile([P, D], mybir.dt.float32)
    psb = psum.tile([P, D], mybir.dt.float32)
    for c in range(nchunks):
        cs = slice(c * CHUNK, min((c + 1) * CHUNK, D))
        nc.tensor.matmul(psb[:, cs], lhsT=ones[:, :], rhs=bias_sb[:1, cs],
                         start=True, stop=True)
    nc.vector.tensor_copy(bias_bcast[:], psb[:])

    with tc.For_i(0, rep, 1):
        _blocks_loop(tc, nc, iv, rows, tpool, opool, bias_bcast,
                     idx_ap, val_ap, w_ap, out_ap)


def _blocks_loop(tc, nc, iv, rows, tpool, opool, bias_bcast,
                 idx_ap, val_ap, w_ap, out_ap):
    with tc.For_i(0, ROWS, G * P) as row0:
        for blk in range(G):
            bs = bass.ds(row0 + blk * P, P)
            idxb = iv.tile([P, K], mybir.dt.int32)
            nc.sync.dma_start(out=idxb[:], in_=idx_ap[bs, :])
            valb = iv.tile([P, K], mybir.dt.float32)
            nc.sync.dma_start(out=valb[:], in_=val_ap[bs, :])

            outb = opool.tile([P, D], mybir.dt.float32)
            nc.vector.tensor_copy(outb[:], bias_bcast[:])

            for k in range(K):
                r = rows.tile([P, D], mybir.dt.float32)
                nc.gpsimd.indirect_dma_start(
                    out=r[:],
                    out_offset=None,
                    in_=w_ap[:],
                    in_offset=bass.IndirectOffsetOnAxis(ap=idxb[:, k:k + 1],
                                                        axis=0),
                )
                # tmp = r * val[:, k] on ACT; acc += tmp on DVE
                tmp = tpool.tile([P, D], mybir.dt.float32, tag="tmp")
                nc.scalar.activation(tmp[:], r[:],
                                     mybir.ActivationFunctionType.Copy,
                                     scale=valb[:, k:k + 1])
                nc.vector.tensor_add(outb[:], outb[:], tmp[:])

            nc.sync.dma_start(out=out_ap[bs, :], in_=outb[:])


def _build(rep=1):
    nc = bacc.Bacc("TRN2", target_bir_lowering=False, debug=False)
    idx_t = nc.dram_tensor("idx", [ROWS, K], mybir.dt.int32,
                           kind="ExternalInput").ap()
    val_t = nc.dram_tensor("val", [ROWS, K], mybir.dt.float32,
                           kind="ExternalInput").ap()
    w_t = nc.dram_tensor("w", [NUM_INPUTS, D], mybir.dt.float32,
                         kind="ExternalInput").ap()
    b_t = nc.dram_tensor("bias", [D], mybir.dt.float32,
                         kind="ExternalInput").ap()
    out_t = nc.dram_tensor("out", [ROWS, D], mybir.dt.float32,
                           kind="ExternalOutput").ap()
    with tile.TileContext(nc) as tc:
        _kernel_body(tc, idx_t, val_t, w_t, b_t, out_t, rep=rep)
    nc.compile()
    return nc


def prepare(feature_indices_0, feature_values_0, feature_indices_1,
            feature_values_1, weight, bias_ft, bias_psqt):
    """Build (cached) program + per-core input maps."""
    idx0 = np.ascontiguousarray(np.asarray(feature_indices_0, dtype=np.int32))
    val0 = np.ascontiguousarray(np.asarray(feature_values_0, dtype=np.float32))
    idx1 = np.ascontiguousarray(np.asarray(feature_indices_1, dtype=np.int32))
    val1 = np.ascontiguousarray(np.asarray(feature_values_1, dtype=np.float32))
    w = np.ascontiguousarray(np.asarray(weight, dtype=np.float32))
    bias = np.concatenate([
        np.asarray(bias_ft, dtype=np.float32).ravel(),
        np.asarray(bias_psqt, dtype=np.float32).ravel(),
    ])

    if "nc" not in _cache:
        _cache["nc"] = _build()
    nc = _cache["nc"]

    in_maps = []
    for c in range(N_CORES):
        sl = slice(c * BPC, (c + 1) * BPC)
        in_maps.append({
            "idx": np.concatenate([idx0[sl], idx1[sl]], axis=0),
            "val": np.concatenate([val0[sl], val1[sl]], axis=0),
            "w": w,
            "bias": bias,
        })
    return nc, in_maps


def kernel(feature_indices_0, feature_values_0, feature_indices_1,
           feature_values_1, weight, bias_ft, bias_psqt):
    global LAST_RESULTS
    nc, in_maps = prepare(feature_indices_0, feature_values_0,
                          feature_indices_1, feature_values_1,
                          weight, bias_ft, bias_psqt)
    res = run_bass_kernel_spmd(nc, in_maps, core_ids=list(range(N_CORES)))
    LAST_RESULTS = res
    outs = [r["out"] for r in res.results]
    out0 = np.concatenate([o[:BPC] for o in outs], axis=0)
    out1 = np.concatenate([o[BPC:] for o in outs], axis=0)
    return out0, out1
